# revision 25
# baseline (speedup 1.0000x reference)
"""Capacity-calibrated partial transport reranker on 8 trn2 NeuronCores.

Math: every step of the reference Sinkhorn-style loop multiplies the plan by a
row vector, a column vector, or a scalar, so the plan stays in factored form
    plan_i = K  *  u_i[:, None]  *  v_i[None, :]
the whole way through.  Each iteration therefore reduces to two matvecs with
the fixed Gibbs kernel K (sharded over columns: 512 per core), one tiny
AllGather of the row-sum partials (+ deferred total-mass scalar), and O(N/8)
vector work.  The f32 iteration reaches an exact fixed point by ~iter 10
(verified numerically: iterates 10..50 are bit-identical), so NITERS < 50
iterations reproduce the 50-iteration reference to ~1e-7.
"""

import numpy as np

M, N, D = 1024, 4096, 128
NCORES = 8
NL = N // NCORES          # 512 columns per core
MT = M // 128             # 8 m-blocks
NT = NL // 128            # 4 local n-blocks
NITERS = 14
EPS = 0.05
STRENGTH = 0.5
TINY = 1e-12

PAYC = 9                  # payload cols: 8 Kv-Mvec cols + 1 t-slot


def _emit(nc, tc, io):
    import os
    from concourse import bass, mybir
    from concourse.bass import ts

    KSTAGE = int(os.environ.get("KSTAGE", "99"))

    f32 = mybir.dt.float32
    AX = mybir.AxisListType
    OP = mybir.AluOpType
    AF = mybir.ActivationFunctionType

    user_d, item_d, a_d, bl_d, bf_d, mb_d, id_d, onc_d, onr_d, plan_d, usage_d, parts_d = io

    # ---------------- persistent SBUF ----------------
    from contextlib import ExitStack
    pools = ExitStack()
    persist = pools.enter_context(tc.tile_pool(name="persist", bufs=1))

    def T(shape, name):
        return persist.tile(shape, f32, name=name, tag=name)

    ident = T([128, 128], "ident")
    onescol = T([128, 1], "onescol")
    onesrow = T([1, 128], "onesrow")
    um = T([128, MT * 128], "um")      # user feats, m-blocks
    vm = T([128, NT * 128], "vm")      # item feats (local shard)
    umT = T([128, MT * 128], "umT")    # [d, m]
    vmT = T([128, NT * 128], "vmT")    # [d, n]
    u2 = T([128, MT], "u2")            # |u|^2, Mvec
    v2bc = T([128, NL], "v2bc")        # |v|^2 bcast over partitions
    penbc = T([128, NL], "penbc")      # penalty bcast
    Csb = [T([128, NL], f"C{t}") for t in range(MT)]
    Ksb = [T([128, NL], f"K{t}") for t in range(MT)]
    KT = [T([128, M], f"KT{t}") for t in range(NT)]   # [n, m]
    KC = [T([128, NL], f"KC{t}") for t in range(MT)]
    a_sb = T([128, MT], "a_sb")        # source_mass Mvec
    bl_sb = T([128, NT], "bl_sb")      # local capacity Nvec
    blrow = T([1, NL], "blrow")
    bfull = T([128, N // 128], "bfull")
    mbs = T([1, 1], "mbs")
    u_sb = T([128, MT], "u_sb")
    v_sb = T([128, NT], "v_sb")
    pay = T([128, PAYC], "pay")
    agin2 = T([128, NCORES * PAYC], "agin2")
    ssum = T([128, PAYC], "ssum")      # rank-summed payload
    sbc = T([128, 1], "sbc")           # s broadcast per-partition
    scratch = T([128, NL], "scratch")
    scrM = T([128, MT], "scrM")
    scrN = T([128, NT], "scrN")
    scrN2 = T([128, NT], "scrN2")
    gam = T([128, NT], "gam")
    usage = T([128, NT], "usage")
    tiny1 = T([1, 8], "tiny1")         # misc 1-partition scratch
    vrow = T([1, NL], "vrow")
    vbc = T([128, NL], "vbc")
    parts_sb = T([1, 2], "parts_sb")
    mx8 = T([1, 64], "mx8")

    ps_big = pools.enter_context(tc.tile_pool(name="ps_big", bufs=2, space="PSUM"))
    ps_tr = pools.enter_context(tc.tile_pool(name="ps_tr", bufs=2, space="PSUM"))
    ps_vec = pools.enter_context(tc.tile_pool(name="ps_vec", bufs=2, space="PSUM"))
    ps_tiny = pools.enter_context(tc.tile_pool(name="ps_tiny", bufs=1, space="PSUM"))
    dram = pools.enter_context(tc.tile_pool(name="dram", bufs=2, space="DRAM"))
    planp = pools.enter_context(tc.tile_pool(name="planp", bufs=3))

    RG = [list(range(NCORES))]

    # ---------------- input DMAs ----------------
    nc.sync.dma_start(out=ident[:], in_=id_d[:, :])
    nc.sync.dma_start(out=onescol[:], in_=onc_d[:, :])
    nc.sync.dma_start(out=onesrow[:], in_=onr_d[:, :])
    for t in range(MT):
        nc.sync.dma_start(out=um[:, ts(t, 128)], in_=user_d[ts(t, 128), :])
    for t in range(NT):
        nc.sync.dma_start(out=vm[:, ts(t, 128)], in_=item_d[ts(t, 128), :])
    nc.sync.dma_start(out=a_sb[:], in_=a_d.ap().rearrange("(t p) -> p t", p=128))
    nc.sync.dma_start(out=bl_sb[:], in_=bl_d.ap().rearrange("(t p) -> p t", p=128))
    nc.sync.dma_start(out=blrow[:], in_=bl_d.ap().rearrange("(o n) -> o n", o=1))
    nc.sync.dma_start(out=bfull[:], in_=bf_d.ap().rearrange("(p t) -> p t", p=128))
    nc.sync.dma_start(out=mbs[:], in_=mb_d[:, :])

    nc.vector.memset(u_sb[:], 1.0)
    nc.vector.memset(v_sb[:], 1.0)
    nc.vector.memset(pay[:, 8:9], 0.0)
    nc.vector.memset(tiny1[:], 0.0)

    # ---------------- transposes of feature blocks ----------------
    for t in range(MT):
        pt = ps_tr.tile([128, 128], f32, tag="ptr")
        nc.tensor.transpose(pt[:], um[:, ts(t, 128)], ident[:])
        nc.scalar.copy(umT[:, ts(t, 128)], pt[:])
    for t in range(NT):
        pt = ps_tr.tile([128, 128], f32, tag="ptr")
        nc.tensor.transpose(pt[:], vm[:, ts(t, 128)], ident[:])
        nc.scalar.copy(vmT[:, ts(t, 128)], pt[:])

    # u2 (Mvec) via Square-activation with free-sum accumulator
    for t in range(MT):
        nc.scalar.activation(scratch[:, 0:128], um[:, ts(t, 128)], AF.Square,
                             accum_out=u2[:, t:t + 1])
    # v2 as a row, then broadcast across partitions
    nc.scalar.activation(scratch[:, 0:NL], vmT[:], AF.Square)
    p_v2 = ps_tiny.tile([1, NL], f32, tag="pt1")
    nc.tensor.matmul(p_v2[:], onescol[:], scratch[:, 0:NL], start=True, stop=True)
    nc.scalar.copy(vrow[:], p_v2[:])                    # temp: v2 row in vrow
    p_bc = ps_big.tile([128, NL], f32, tag="pbig")
    nc.tensor.matmul(p_bc[:], onesrow[:], vrow[:], start=True, stop=True)
    nc.scalar.copy(v2bc[:], p_bc[:])

    # penalty row: STRENGTH * (1 - b / (max(b) + TINY)), then broadcast
    nc.vector.reduce_max(out=scrM[:, 0:1], in_=bfull[:], axis=AX.X)
    pmx = ps_tr.tile([1, 128], f32, tag="ptr")
    nc.tensor.transpose(pmx[0:1, :], scrM[:, 0:1], ident[:])
    nc.vector.reduce_max(out=tiny1[0:1, 0:1], in_=pmx[0:1, :], axis=AX.X)
    nc.vector.tensor_scalar(tiny1[0:1, 1:2], tiny1[0:1, 0:1], TINY, None, op0=OP.add)
    nc.vector.reciprocal(tiny1[0:1, 2:3], tiny1[0:1, 1:2])
    nc.scalar.mul(tiny1[0:1, 3:4], tiny1[0:1, 2:3], -STRENGTH)
    # penrow = b_l * (-S/maxb) + S
    nc.vector.tensor_scalar(vrow[:], blrow[:], tiny1[0:1, 3:4], STRENGTH,
                            op0=OP.mult, op1=OP.add)
    p_bc = ps_big.tile([128, NL], f32, tag="pbig")
    nc.tensor.matmul(p_bc[:], onesrow[:], vrow[:], start=True, stop=True)
    nc.scalar.copy(penbc[:], p_bc[:])

    # ---------------- raw cost tiles + local max ----------------
    for t in range(MT):
        p_c = ps_big.tile([128, NL], f32, tag="pbig")
        nc.tensor.matmul(p_c[:], umT[:, ts(t, 128)], vmT[:], start=True, stop=True)
        # raw = -2*uv + u2[m]  (ACT), then += v2[n] (DVE)
        nc.scalar.activation(Csb[t][:], p_c[:], AF.Identity, scale=-2.0,
                             bias=u2[:, t:t + 1])
        nc.vector.tensor_tensor(Csb[t][:], Csb[t][:], v2bc[:], op=OP.add)
        nc.vector.reduce_max(out=scrM[:, t:t + 1], in_=Csb[t][:], axis=AX.X)
    nc.vector.reduce_max(out=scrM[:, 0:1], in_=scrM[:], axis=AX.X)
    pmx = ps_tr.tile([1, 128], f32, tag="ptr")
    nc.tensor.transpose(pmx[0:1, :], scrM[:, 0:1], ident[:])
    nc.vector.reduce_max(out=tiny1[0:1, 4:5], in_=pmx[0:1, :], axis=AX.X)

    # AG#0: global max of raw cost
    ag0_in = dram.tile([1, 8], f32, tag="ag0i")
    ag0_out = dram.tile([1, 64], f32, tag="ag0o")
    nc.sync.dma_start(out=ag0_in[:], in_=tiny1[0:1, 0:8])   # col4 = local max, rest harmless
    nc.gpsimd.collective_compute(
        "AllGather", OP.bypass, replica_groups=RG,
        ins=[ag0_in[:].opt()], outs=[ag0_out[:].opt()])
    nc.sync.dma_start(out=mx8[:], in_=ag0_out[:])
    # NOTE: cols != 4 of each rank's block contain that rank's tiny1[0:4] values
    # (maxb pieces etc.) — all of these are <= the true cost max? Not guaranteed.
    # To be safe we only reduce the stride-8 slice at offset 4.
    nc.vector.reduce_max(out=tiny1[0:1, 0:1],
                         in_=mx8[:].rearrange("o (r c) -> o c r", c=8)[:, 4],
                         axis=AX.X)
    nc.vector.tensor_scalar(tiny1[0:1, 1:2], tiny1[0:1, 0:1], TINY, None, op0=OP.add)
    nc.vector.reciprocal(tiny1[0:1, 2:3], tiny1[0:1, 1:2])  # 1/(maxc+TINY)
    p_imc = ps_tiny.tile([128, 1], f32, tag="pt1")
    nc.tensor.matmul(p_imc[0:128, 0:1], onesrow[:], tiny1[0:1, 2:3], start=True, stop=True)
    nc.scalar.copy(sbc[:], p_imc[:, 0:1])               # sbc temporarily = 1/maxc bcast

    if KSTAGE <= 1:
        nc.sync.dma_start(out=parts_d[0:1, 0:1], in_=sbc[0:1, 0:1])
        pools.close()
        return

    # ---------------- C, K, KC, KT, row sums ----------------
    for t in range(MT):
        # C = relu(raw) * inv_maxc + pen
        nc.vector.tensor_scalar(Csb[t][:], Csb[t][:], 0.0, sbc[:, 0:1],
                                op0=OP.max, op1=OP.mult)
        nc.vector.tensor_tensor(Csb[t][:], Csb[t][:], penbc[:], op=OP.add)
        nc.scalar.activation(Ksb[t][:], Csb[t][:], AF.Exp, scale=-1.0 / EPS)
        nc.vector.tensor_tensor(KC[t][:], Ksb[t][:], Csb[t][:], op=OP.mult)
        # local row-sums of K -> payload Mvec col t (Kv with v=1)
        nc.vector.reduce_sum(out=pay[:, t:t + 1], in_=Ksb[t][:], axis=AX.X)
    # KT blocks via PE transpose
    for tm in range(MT):
        for tn in range(NT):
            pt = ps_tr.tile([128, 128], f32, tag="ptr")
            nc.tensor.transpose(pt[:], Ksb[tm][:, ts(tn, 128)], ident[:])
            nc.scalar.copy(KT[tn][:, ts(tm, 128)], pt[:])
    # sumK partial -> t-slot
    nc.vector.reduce_sum(out=scrM[:, 0:1], in_=pay[:, 0:8], axis=AX.X)
    p_t = ps_tiny.tile([1, 1], f32, tag="pt1")
    nc.tensor.matmul(p_t[0:1, 0:1], scrM[:, 0:1], onescol[:], start=True, stop=True)
    nc.scalar.copy(pay[0:1, 8:9], p_t[0:1, 0:1])

    # ---------------- payload exchange helper ----------------
    def send_payload():
        agi = dram.tile([PAYC, 128], f32, tag="agi")       # column-major flat (c p)
        ago = dram.tile([NCORES, PAYC * 128], f32, tag="ago")
        nc.sync.dma_start(out=agi[:].rearrange("c p -> p c"), in_=pay[:])
        nc.gpsimd.collective_compute(
            "AllGather", OP.bypass, replica_groups=RG,
            ins=[agi[:].opt()], outs=[ago[:].opt()])
        # agin2[p, r*9+c] = ago[r, c*128+p]
        nc.sync.dma_start(
            out=agin2[:].rearrange("p (r c) -> p r c", c=PAYC),
            in_=ago[:].rearrange("r (c p) -> p r c", p=128))
        # rank sum: ssum[p, c] = sum_r agin2[p, r*9+c]
        nc.vector.reduce_sum(out=ssum[:],
                             in_=agin2[:].rearrange("p (r c) -> p c r", c=PAYC),
                             axis=AX.X)

    def u_update():
        """s = mb/t ; rho = min(a / (s*u*Kv), 1) ; u *= rho*s   (all from ssum)."""
        nc.scalar.copy(tiny1[0:1, 0:1], ssum[0:1, 8:9])        # t_glob
        nc.vector.reciprocal(tiny1[0:1, 1:2], tiny1[0:1, 0:1])
        nc.vector.tensor_tensor(tiny1[0:1, 2:3], tiny1[0:1, 1:2], mbs[:], op=OP.mult)
        p_s = ps_tiny.tile([128, 1], f32, tag="pt1")
        nc.tensor.matmul(p_s[0:128, 0:1], onesrow[:], tiny1[0:1, 2:3], start=True, stop=True)
        nc.scalar.copy(sbc[:], p_s[:, 0:1])
        # rs = (Kv ∘ sbc) ∘ u  -> scrM
        nc.vector.scalar_tensor_tensor(scrM[:], ssum[:, 0:8], sbc[:, 0:1], u_sb[:],
                                       op0=OP.mult, op1=OP.mult)
        nc.vector.reciprocal(scrM[:], scrM[:])
        nc.vector.tensor_tensor(scrM[:], a_sb[:], scrM[:], op=OP.mult)   # a/rs
        nc.vector.tensor_scalar(scrM[:], scrM[:], 1.0, sbc[:, 0:1],
                                op0=OP.min, op1=OP.mult)                  # rho*s
        nc.vector.tensor_tensor(u_sb[:], u_sb[:], scrM[:], op=OP.mult)

    def ktu_matvec(dst_psum, Kblocks):
        for tn in range(NT):
            for tm in range(MT):
                nc.tensor.matmul(dst_psum[:, tn:tn + 1],
                                 Kblocks[tm][:, ts(tn, 128)],
                                 u_sb[:, tm:tm + 1],
                                 start=(tm == 0), stop=(tm == MT - 1))

    # ---------------- Sinkhorn iterations ----------------
    send_payload()
    if KSTAGE <= 2:
        nc.sync.dma_start(out=parts_d[0:1, 0:1], in_=ssum[0:1, 8:9])
        pools.close()
        return
    n_iters = 1 if KSTAGE in (3, 31, 32, 33, 34, 35, 36, 37) else NITERS
    for it in range(n_iters):
        u_update()
        if KSTAGE == 31:
            break
        p_ktu = ps_vec.tile([128, NT], f32, tag="pvec")
        ktu_matvec(p_ktu, Ksb)
        if KSTAGE == 34:
            nc.scalar.copy(scrN[:], p_ktu[:])
            nc.vector.tensor_copy(u_sb[:, 0:NT], scrN[:])
            break
        if KSTAGE == 35:
            nc.vector.tensor_tensor(scrN[:], v_sb[:], p_ktu[:], op=OP.mult)
            nc.vector.tensor_copy(u_sb[:, 0:NT], scrN[:])
            break
        # c = v*KTu ; v *= min(b/c, 1) ; tpart = sum(v_new*KTu)
        nc.vector.tensor_tensor(scrN[:], v_sb[:], p_ktu[:], op=OP.mult)   # c
        nc.vector.reciprocal(scrN2[:], scrN[:])
        nc.vector.tensor_tensor(scrN2[:], bl_sb[:], scrN2[:], op=OP.mult)  # b/c
        nc.vector.scalar_tensor_tensor(v_sb[:], scrN2[:], 1.0, v_sb[:],
                                       op0=OP.min, op1=OP.mult)
        if KSTAGE == 37:
            nc.vector.tensor_copy(u_sb[:, 0:NT], v_sb[:])
            break
        nc.vector.tensor_tensor(scrN[:], v_sb[:], p_ktu[:], op=OP.mult)
        nc.vector.reduce_sum(out=scrM[:, 0:1], in_=scrN[:], axis=AX.X)
        if KSTAGE == 36:
            break
        p_t = ps_tiny.tile([1, 1], f32, tag="pt1")
        nc.tensor.matmul(p_t[0:1, 0:1], scrM[:, 0:1], onescol[:], start=True, stop=True)
        nc.scalar.copy(pay[0:1, 8:9], p_t[0:1, 0:1])
        if KSTAGE == 32:
            break
        # Kv partials (Mvec) with new v
        p_kv = ps_vec.tile([128, MT], f32, tag="pvec")
        for tm in range(MT):
            for tn in range(NT):
                nc.tensor.matmul(p_kv[:, tm:tm + 1],
                                 KT[tn][:, ts(tm, 128)],
                                 v_sb[:, tn:tn + 1],
                                 start=(tn == 0), stop=(tn == NT - 1))
        nc.scalar.copy(pay[:, 0:8], p_kv[:])
        if KSTAGE == 33:
            break
        send_payload()

    if KSTAGE <= 4 or KSTAGE in (31, 32, 33, 34, 35, 36, 37):
        nc.sync.dma_start(out=parts_d[0:1, 0:1], in_=u_sb[0:1, 0:1])
        pools.close()
        return

    # ---------------- epilogue: final feasibility clip + outputs ----------------
    u_update()                                   # final row clip (u_fin)
    p_ktu = ps_vec.tile([128, NT], f32, tag="pvec")
    ktu_matvec(p_ktu, Ksb)
    nc.vector.tensor_tensor(scrN[:], v_sb[:], p_ktu[:], op=OP.mult)        # c
    nc.vector.reciprocal(scrN2[:], scrN[:])
    nc.vector.tensor_tensor(scrN2[:], bl_sb[:], scrN2[:], op=OP.mult)      # b/c
    nc.vector.tensor_scalar(gam[:], scrN2[:], 1.0, None, op0=OP.min)       # gamma
    nc.vector.tensor_tensor(v_sb[:], v_sb[:], gam[:], op=OP.mult)          # v_fin
    # usage = c * gamma ; tmass partial = sum(usage)
    nc.vector.tensor_tensor(usage[:], scrN[:], gam[:], op=OP.mult)
    nc.vector.reduce_sum(out=scrM[:, 0:1], in_=usage[:], axis=AX.X)
    p_tm = ps_tiny.tile([1, 1], f32, tag="pt1")
    nc.tensor.matmul(p_tm[0:1, 0:1], scrM[:, 0:1], onescol[:], start=True, stop=True)
    nc.scalar.copy(parts_sb[0:1, 0:1], p_tm[0:1, 0:1])
    nc.sync.dma_start(out=usage_d.ap().rearrange("(t p) -> p t", p=128), in_=usage[:])

    # score partial = sum_n v_fin * (KC^T u_fin)
    p_kc = ps_vec.tile([128, NT], f32, tag="pvec")
    ktu_matvec(p_kc, KC)
    nc.vector.tensor_tensor(scrN[:], v_sb[:], p_kc[:], op=OP.mult)
    nc.vector.reduce_sum(out=scrM[:, 0:1], in_=scrN[:], axis=AX.X)
    p_sp = ps_tiny.tile([1, 1], f32, tag="pt1")
    nc.tensor.matmul(p_sp[0:1, 0:1], scrM[:, 0:1], onescol[:], start=True, stop=True)
    nc.scalar.copy(parts_sb[0:1, 1:2], p_sp[0:1, 0:1])
    nc.sync.dma_start(out=parts_d[:, :], in_=parts_sb[:])

    # v_fin as a broadcast row for plan materialization
    for tn in range(NT):
        pt = ps_tr.tile([1, 128], f32, tag="ptr")
        nc.tensor.transpose(pt[0:1, :], v_sb[:, tn:tn + 1], ident[:])
        nc.scalar.copy(vrow[0:1, ts(tn, 128)], pt[0:1, :])
    p_bc = ps_big.tile([128, NL], f32, tag="pbig")
    nc.tensor.matmul(p_bc[:], onesrow[:], vrow[:], start=True, stop=True)
    nc.scalar.copy(vbc[:], p_bc[:])

    # plan tiles: K * u_fin[m] * v_fin[n]
    for tm in range(MT):
        ptile = planp.tile([128, NL], f32, tag="ptile")
        nc.scalar.activation(ptile[:], Ksb[tm][:], AF.Copy, scale=u_sb[:, tm:tm + 1])
        nc.vector.tensor_tensor(ptile[:], ptile[:], vbc[:], op=OP.mult)
        nc.sync.dma_start(out=plan_d[ts(tm, 128), :], in_=ptile[:])

    pools.close()


def _build():
    import sys
    if "/opt/trn_rl_repo" not in sys.path:
        sys.path.insert(0, "/opt/trn_rl_repo")
    from concourse import bacc, mybir, tile

    f32 = mybir.dt.float32
    nc = bacc.Bacc("TRN2", target_bir_lowering=False, debug=False,
                   enable_asserts=False, num_devices=NCORES)
    user_d = nc.dram_tensor("user_nodes", [M, D], f32, kind="ExternalInput")
    item_d = nc.dram_tensor("item_l", [NL, D], f32, kind="ExternalInput")
    a_d = nc.dram_tensor("source_mass", [M], f32, kind="ExternalInput")
    bl_d = nc.dram_tensor("cap_l", [NL], f32, kind="ExternalInput")
    bf_d = nc.dram_tensor("cap_full", [N], f32, kind="ExternalInput")
    mb_d = nc.dram_tensor("mass_budget", [1, 1], f32, kind="ExternalInput")
    id_d = nc.dram_tensor("ident", [128, 128], f32, kind="ExternalInput")
    onc_d = nc.dram_tensor("ones_col", [128, 1], f32, kind="ExternalInput")
    onr_d = nc.dram_tensor("ones_row", [1, 128], f32, kind="ExternalInput")
    plan_d = nc.dram_tensor("plan_l", [M, NL], f32, kind="ExternalOutput")
    usage_d = nc.dram_tensor("usage_l", [NL], f32, kind="ExternalOutput")
    parts_d = nc.dram_tensor("partials", [1, 2], f32, kind="ExternalOutput")
    io = (user_d, item_d, a_d, bl_d, bf_d, mb_d, id_d, onc_d, onr_d,
          plan_d, usage_d, parts_d)
    with tile.TileContext(nc) as tc:
        _emit(nc, tc, io)
    nc.compile()
    return nc


_NC_CACHE = None


def _get_nc():
    global _NC_CACHE
    if _NC_CACHE is None:
        _NC_CACHE = _build()
    return _NC_CACHE


def _in_maps(user_nodes, item_nodes, source_mass, target_capacity, mass_budget):
    f = np.float32
    user_nodes = np.ascontiguousarray(user_nodes, dtype=f)
    item_nodes = np.ascontiguousarray(item_nodes, dtype=f)
    source_mass = np.ascontiguousarray(source_mass, dtype=f)
    target_capacity = np.ascontiguousarray(target_capacity, dtype=f)
    mb = np.array(mass_budget, dtype=f).reshape(1, 1)
    ident = np.eye(128, dtype=f)
    onescol = np.ones((128, 1), dtype=f)
    onesrow = np.ones((1, 128), dtype=f)
    maps = []
    for c in range(NCORES):
        maps.append({
            "user_nodes": user_nodes,
            "item_l": np.ascontiguousarray(item_nodes[c * NL:(c + 1) * NL]),
            "source_mass": source_mass,
            "cap_l": np.ascontiguousarray(target_capacity[c * NL:(c + 1) * NL]),
            "cap_full": target_capacity,
            "mass_budget": mb,
            "ident": ident,
            "ones_col": onescol,
            "ones_row": onesrow,
        })
    return maps


def _run(in_maps, trace=False, trace_cores=None):
    import sys
    if "/opt/trn_rl_repo" not in sys.path:
        sys.path.insert(0, "/opt/trn_rl_repo")
    from concourse import bass_utils
    nc = _get_nc()
    return bass_utils.run_bass_kernel_spmd(
        nc, in_maps, core_ids=list(range(NCORES)),
        trace=trace, trace_cores=trace_cores)


def _assemble(results):
    plan = np.concatenate(
        [results[c]["plan_l"].reshape(M, NL) for c in range(NCORES)], axis=1)
    usage = np.concatenate(
        [results[c]["usage_l"].reshape(NL) for c in range(NCORES)], axis=0)
    parts = np.stack([results[c]["partials"].reshape(2) for c in range(NCORES)])
    tmass = np.float32(np.sum(parts[:, 0], dtype=np.float64))
    score = np.float32(-np.sum(parts[:, 1], dtype=np.float64))
    return score, plan, tmass, usage


def kernel(user_nodes, item_nodes, source_mass, target_capacity, mass_budget):
    maps = _in_maps(user_nodes, item_nodes, source_mass, target_capacity, mass_budget)
    res = _run(maps)
    return _assemble(res.results)
